# revision 23
# baseline (speedup 1.0000x reference)
"""Multi-head attention forward (B=4, N=2048, C=1024, H=16) on 8 Trainium2 cores.

Sharding: (batch, head-half) across 8 cores. Core c handles batch b = c//2 and
heads g*8..g*8+8 where g = c%2. Each core computes qkv for its head slice,
attention for its 8 heads, and a partial output projection over its 512
input-channel slice. The host sums the two partial projections per batch
(the tensor-parallel all-reduce) and adds b_proj.

Pipeline (per core), engineered so ScalarE (the exp engine, ~18us per head
pair) and the PE (~19us per head pair) overlap from ~70us onward:

  pass A   k and v for ALL query blocks (PE dense, x double-buffered via DMA).
           k is stored f32r [128, N] per head pair; v bf16 in natural
           [key, d] layout with a fused ones column per head (so P@V also
           produces softmax denominators).
  q        computed per query block ([128, 512] f32r, zero-padded per head),
           interleaved into the attention stream of the previous block, with
           its x chunks re-fetched from DRAM (x is streamed twice overall).
  attn     S^T[key, q] = kT.T @ qz per 128-key chunk (f32r, full PE rows via
           the zero-padding); exp on ScalarE (scale 1/sqrt(hd) folded in,
           bf16 out, pure-Exp stream so the activation table never reloads);
           P^T @ V in bf16 accumulating over key chunks.
  norm     per pair: copy the two PSUM denominator rows into one [2, 512]
           tile, one DVE iterative reciprocal, a tiny ones-matmul broadcast,
           one [128, 512] DVE multiply -> outHT. All off the critical path
           (outHT is consumed one block later).
  proj     f32r, for block nb-1, interleaved 2 cout-chunks per pair into
           block nb's attention stream (dense PE, spread-out yT DMA).
"""

import sys

if "/opt/trn_rl_repo" not in sys.path:
    sys.path.insert(0, "/opt/trn_rl_repo")

import numpy as np

B, N, C = 4, 2048, 1024
H, HD = 16, 64
NCORES = 8
HLOC = H // 2          # heads per core
PAIRS = HLOC // 2      # head-pair tiles per core
CIN = HLOC * HD        # 512: proj input slice per core
NQB = 512              # query-block width
NBLK = N // NQB        # 4
CCH = C // 128         # 8 contraction chunks for the projections
KCH = N // 128         # 16 key chunks

MM_DT_NAME = "float32r"    # qkv + scores + proj matmul dtype
PV_DT_NAME = "bfloat16"    # v and exp(S) tiles (the P@V matmul dtype)

_BUILD_CACHE = {}


def _build(key):
    import concourse.mybir as mybir
    import concourse.tile as tile
    from concourse import bacc

    DT = getattr(mybir.dt, MM_DT_NAME)
    AT = getattr(mybir.dt, PV_DT_NAME)
    F32 = mybir.dt.float32
    AF = mybir.ActivationFunctionType

    nc = bacc.Bacc(None, target_bir_lowering=False)
    xT = nc.dram_tensor("xT", [C, N], DT, kind="ExternalInput")
    wqT = nc.dram_tensor("wqT", [C, CIN], DT, kind="ExternalInput")
    wkT = nc.dram_tensor("wkT", [C, CIN], DT, kind="ExternalInput")
    wvT = nc.dram_tensor("wvT", [C, CIN], DT, kind="ExternalInput")
    wpT = nc.dram_tensor("wpT", [CIN, C], DT, kind="ExternalInput")
    yT = nc.dram_tensor("yT", [C, N], F32, kind="ExternalOutput")

    with nc.allow_low_precision(reason="softmax intermediates kept in bf16"):
        with tile.TileContext(nc) as tc:
            _emit(nc, tc, tile, mybir, DT, AT, F32, AF, xT, wqT, wkT, wvT, wpT, yT)
    nc.compile()
    return nc


def _emit(nc, tc, tile, mybir, DT, AT, F32, AF, xT, wqT, wkT, wvT, wpT, yT):
    from contextlib import ExitStack

    ctx = ExitStack()
    with ctx:
        persist = ctx.enter_context(tc.tile_pool(name="persist", bufs=1))
        # wk chunks in pass A, then exp tiles (same 2KB/partition slot size)
        rot = ctx.enter_context(tc.tile_pool(name="rot", bufs=8))
        xpool = ctx.enter_context(tc.tile_pool(name="xpool", bufs=12))
        qpool = ctx.enter_context(tc.tile_pool(name="qpool", bufs=2))
        small = ctx.enter_context(tc.tile_pool(name="small", bufs=2))
        pvpool = ctx.enter_context(tc.tile_pool(name="pvpool", bufs=2))
        ypool = ctx.enter_context(tc.tile_pool(name="ypool", bufs=3))
        outs = ctx.enter_context(tc.tile_pool(name="outs", bufs=2))
        ps_s = ctx.enter_context(tc.tile_pool(name="ps_s", bufs=2, space="PSUM"))
        ps_v = ctx.enter_context(tc.tile_pool(name="ps_v", bufs=2, space="PSUM"))
        ps_acc = ctx.enter_context(tc.tile_pool(name="ps_acc", bufs=2, space="PSUM"))

        # --- persistent tiles ---------------------------------------------
        kT = [persist.tile([128, N], DT, tag=f"kT{p}", name=f"kT{p}") for p in range(PAIRS)]
        v_sb = [persist.tile([128, (HLOC + 1) * (HD + 1)], AT, tag=f"v{kc}", name=f"v{kc}") for kc in range(KCH)]
        wq_sb = [persist.tile([128, CIN], DT, tag=f"wq{ci}", name=f"wq{ci}") for ci in range(CCH)]
        wv_sb = [persist.tile([128, CIN], DT, tag=f"wv{ci}", name=f"wv{ci}") for ci in range(CCH)]
        wp_sb = [persist.tile([128, C], DT, tag=f"wp{pc}", name=f"wp{pc}") for pc in range(CIN // 128)]
        ones_m = persist.tile([1, HD], DT, tag="ones_m")  # bc-matmul stationary
        ones_f32 = persist.tile([128, HLOC], F32, tag="ones_f32")

        nc.vector.memset(ones_f32[:], 1.0)
        nc.vector.tensor_copy(ones_m[:], ones_f32[0:1, 0:1].broadcast_to((1, HD)))
        for kc in range(KCH):
            v3 = v_sb[kc][:, 0:HLOC * (HD + 1)].rearrange("p (h d) -> p h d", h=HLOC)
            nc.vector.tensor_copy(v3[:, :, HD], ones_f32[:, 0:HLOC])
            # zero tail pad so head 7's 128-wide stationary window reads zeros
            nc.vector.memset(v_sb[kc][:, HLOC * (HD + 1):].bitcast(mybir.dt.uint16), 0)

        def x_fetch(nb):
            xt = []
            for ci in range(CCH):
                t = xpool.tile([128, NQB], DT, tag="xt", name=f"xt{nb}_{ci}")
                nc.sync.dma_start(t[:], xT[ci * 128:(ci + 1) * 128, nb * NQB:(nb + 1) * NQB])
                xt.append(t)
            return xt

        xt_next = x_fetch(0)
        wk_sb = []
        for ci in range(CCH):
            w = rot.tile([128, CIN], DT, tag="rot", name=f"wk{ci}")
            nc.sync.dma_start(w[:], wkT[ci * 128:(ci + 1) * 128, :])
            wk_sb.append(w)
            nc.sync.dma_start(wv_sb[ci][:], wvT[ci * 128:(ci + 1) * 128, :])
        for ci in range(CCH):
            nc.sync.dma_start(wq_sb[ci][:], wqT[ci * 128:(ci + 1) * 128, :])
        for pc in range(CIN // 128):
            nc.sync.dma_start(wp_sb[pc][:], wpT[pc * 128:(pc + 1) * 128, :])

        # --- pass A: k and v for all query blocks -------------------------
        for nb in range(NBLK):
            nsl = slice(nb * NQB, (nb + 1) * NQB)
            xt = xt_next
            if nb + 1 < NBLK:
                xt_next = x_fetch(nb + 1)
            for dt_i in range(PAIRS):
                acc = ps_acc.tile([128, NQB], F32, tag="acc")
                for ci in range(CCH):
                    nc.tensor.matmul(
                        acc[:], wk_sb[ci][:, dt_i * 128:(dt_i + 1) * 128], xt[ci][:],
                        start=(ci == 0), stop=(ci == CCH - 1),
                    )
                nc.vector.tensor_copy(kT[dt_i][:, nsl], acc[:])
            for j in range(NQB // 128):
                kc = nb * (NQB // 128) + j
                acc = ps_acc.tile([128, CIN], F32, tag="acc")
                for ci in range(CCH):
                    nc.tensor.matmul(
                        acc[:], xt[ci][:, j * 128:(j + 1) * 128], wv_sb[ci][:],
                        start=(ci == 0), stop=(ci == CCH - 1),
                    )
                v3 = v_sb[kc][:, 0:HLOC * (HD + 1)].rearrange("p (h d) -> p h d", h=HLOC)
                nc.vector.tensor_copy(
                    v3[:, :, 0:HD],
                    acc[:].rearrange("p (h d) -> p h d", h=HLOC),
                )

        # q tiles are block-local and zero-padded per head (head's 64 dims on
        # its home partitions, zeros elsewhere) so the score matmul can use
        # the full [128, x] kT pair tile as stationary: full PE rows keep the
        # HAM activity monitor from re-throttling the clock. The pads are
        # zeroed once per ring buffer; in-loop writes only touch home rows.
        for gen in range(2):
            for h in range(HLOC):
                t = qpool.tile([128, NQB], DT, tag=f"qz{h}", name=f"qz{h}")
                pad = slice(64, 128) if h % 2 == 0 else slice(0, 64)
                nc.vector.memset(t[pad, :].bitcast(mybir.dt.uint32), 0)

        def q_group(xt, p):
            """q for pair p of one block -> two zero-padded qz tiles."""
            acc = ps_acc.tile([128, NQB], F32, tag="acc")
            for ci in range(CCH):
                nc.tensor.matmul(
                    acc[:], wq_sb[ci][:, p * 128:(p + 1) * 128], xt[ci][:],
                    start=(ci == 0), stop=(ci == CCH - 1),
                )
            ta = qpool.tile([128, NQB], DT, tag=f"qz{2 * p}", name=f"qz{2 * p}")
            tb = qpool.tile([128, NQB], DT, tag=f"qz{2 * p + 1}", name=f"qz{2 * p + 1}")
            nc.vector.tensor_copy(ta[0:64, :], acc[0:64, :])
            nc.vector.tensor_copy(tb[64:128, :], acc[64:128, :])
            return ta, tb

        # q for block 0 (its pass-A x chunks have rotated out; re-fetch)
        xtq = x_fetch(0)
        qz_next = []
        for p in range(PAIRS):
            qz_next.extend(q_group(xtq, p))

        # --- attention + projection, per query block ----------------------
        def proj_emit(nb_prev, outHT_prev):
            nsl_prev = slice(nb_prev * NQB, (nb_prev + 1) * NQB)
            for ct in range(C // 128):
                acc = ps_acc.tile([128, NQB], F32, tag="acc", name="acc")
                for p in range(PAIRS):
                    nc.tensor.matmul(
                        acc[:], wp_sb[p][:, ct * 128:(ct + 1) * 128],
                        outHT_prev[p][:],
                        start=(p == 0), stop=(p == PAIRS - 1),
                    )
                yt = ypool.tile([128, NQB], F32, tag="yt", name="yt")
                nc.vector.tensor_copy(yt[:], acc[:])
                nc.sync.dma_start(yT[ct * 128:(ct + 1) * 128, nsl_prev], yt[:])
                yield

        proj_gen = None
        for nb in range(NBLK):
            qz = qz_next
            if nb + 1 < NBLK:
                xtq = x_fetch(nb + 1)
            outHT = [outs.tile([128, NQB], DT, tag=f"outHT{p}", name=f"outHT{p}") for p in range(PAIRS)]
            qz_next = []
            for p in range(PAIRS):
                pv_a = ps_v.tile([128, NQB], F32, tag="pv", name="pv_a")
                pv_b = ps_v.tile([128, NQB], F32, tag="pv", name="pv_b")
                for kc2 in range(KCH // 2):
                    # issue order: 4x S, 2x exp, 4x V — the PE never has to
                    # wait mid-group on ScalarE (keeps HAM warm)
                    st_et = []
                    for head in range(2):
                        st = ps_s.tile([128, 2 * NQB], F32, tag="st", name="st")
                        et = rot.tile([128, 2 * NQB], AT, tag="rot", name="et")
                        st_et.append((st, et))
                    for half in range(2):
                        kc = kc2 * 2 + half
                        ksl = slice(kc * 128, (kc + 1) * 128)
                        csl = slice(half * NQB, (half + 1) * NQB)
                        for head in range(2):
                            nc.tensor.matmul(
                                st_et[head][0][:, csl],
                                kT[p][:, ksl],
                                qz[2 * p + head][:, :],
                                start=True, stop=True,
                            )
                    for st, et in st_et:
                        nc.scalar.activation(et[:], st[:], AF.Exp, scale=0.125)
                    for head, pv in ((0, pv_a), (1, pv_b)):
                        et = st_et[head][1]
                        vstart = (2 * p + head) * (HD + 1)
                        for half in range(2):
                            kc = kc2 * 2 + half
                            csl = slice(half * NQB, (half + 1) * NQB)
                            nc.tensor.matmul(
                                pv[:], v_sb[kc][:, vstart:vstart + 128], et[:, csl],
                                start=(kc == 0), stop=(kc == KCH - 1),
                            )

                # normalize: PSUM rows 0-63 are out^T, row 64 the denominator.
                # DVE iterative reciprocal straight off the PSUM row (exact,
                # keeps ScalarE a pure-Exp stream so its activation table
                # never reloads), ones-matmul broadcast, DVE multiply.
                for head, pv, rbase in ((0, pv_a, 0), (1, pv_b, 64)):
                    rec = small.tile([1, NQB], F32, tag="rec", name="rec")
                    nc.vector.reciprocal(rec[:], pv[HD:HD + 1, :])
                    rec_dt = small.tile([1, NQB], DT, tag="rec_dt", name="rec_dt")
                    nc.vector.tensor_copy(rec_dt[:], rec[:])
                    pv_sb = pvpool.tile([HD, NQB], F32, tag="pv_sb", name="pv_sb")
                    nc.vector.tensor_copy(pv_sb[:], pv[0:HD, :])
                    bc = ps_acc.tile([HD, NQB], F32, tag="acc", name="bc")
                    nc.tensor.matmul(bc[:], ones_m[:], rec_dt[:], start=True, stop=True)
                    nc.vector.tensor_mul(
                        outHT[p][rbase:rbase + HD, :], pv_sb[:], bc[:],
                    )

                # interleaved work: 2 cout-chunks of block nb-1's projection,
                # then the q matmul group for pair p of block nb+1
                if proj_gen is not None:
                    next(proj_gen, None)
                    next(proj_gen, None)
                if nb + 1 < NBLK:
                    qz_next.extend(q_group(xtq, p))
            proj_gen = proj_emit(nb, outHT)
        for _ in proj_gen:
            pass


def _get_nc():
    key = (MM_DT_NAME, PV_DT_NAME)
    if key not in _BUILD_CACHE:
        _BUILD_CACHE[key] = _build(key)
    return _BUILD_CACHE[key]


def _make_in_maps(np_inputs):
    x = np.asarray(np_inputs["x"], dtype=np.float32)
    W_qkv = np.asarray(np_inputs["W_qkv"], dtype=np.float32)
    W_proj = np.asarray(np_inputs["W_proj"], dtype=np.float32)
    in_maps = []
    for c in range(NCORES):
        b, g = divmod(c, 2)
        rq = slice(g * CIN, (g + 1) * CIN)
        rk = slice(C + g * CIN, C + (g + 1) * CIN)
        rv = slice(2 * C + g * CIN, 2 * C + (g + 1) * CIN)
        in_maps.append({
            "xT": np.ascontiguousarray(x[b].T),
            "wqT": np.ascontiguousarray(W_qkv[rq].T),
            "wkT": np.ascontiguousarray(W_qkv[rk].T),
            "wvT": np.ascontiguousarray(W_qkv[rv].T),
            "wpT": np.ascontiguousarray(W_proj[:, g * CIN:(g + 1) * CIN].T),
        })
    return in_maps


def kernel(x, W_qkv, W_proj, b_proj):
    from concourse import bass_utils

    b_proj = np.asarray(b_proj, dtype=np.float32)
    nc = _get_nc()
    in_maps = _make_in_maps({"x": x, "W_qkv": W_qkv, "W_proj": W_proj})
    res = bass_utils.run_bass_kernel_spmd(nc, in_maps, core_ids=list(range(NCORES)))
    y = np.empty((B, N, C), dtype=np.float32)
    for b in range(B):
        yt = res.results[2 * b]["yT"] + res.results[2 * b + 1]["yT"]
        y[b] = yt.T
    return y + b_proj[None, None, :]


# revision 25
# speedup vs baseline: 1.1784x; 1.1784x over previous
"""Multi-head attention forward (B=4, N=2048, C=1024, H=16) on 8 Trainium2 cores.

Sharding: (batch, head-half) across 8 cores. Core c handles batch b = c//2 and
heads g*8..g*8+8 where g = c%2. Each core computes qkv for its head slice,
attention for its 8 heads, and a partial output projection over its 512
input-channel slice. The host sums the two partial projections per batch
(the tensor-parallel all-reduce) and adds b_proj.

Pipeline (per core), engineered so ScalarE (the exp engine, ~18us per head
pair) and the PE (~19us per head pair) overlap from ~70us onward:

  pass A   k and v for ALL query blocks (PE dense, x double-buffered via DMA).
           k is stored f32r [128, N] per head pair; v bf16 in natural
           [key, d] layout with a fused ones column per head (so P@V also
           produces softmax denominators).
  q        computed per query block ([128, 512] f32r, zero-padded per head),
           interleaved into the attention stream of the previous block, with
           its x chunks re-fetched from DRAM (x is streamed twice overall).
  attn     S^T[key, q] = kT.T @ qz per 128-key chunk (f32r, full PE rows via
           the zero-padding); exp on ScalarE (scale 1/sqrt(hd) folded in,
           bf16 out, pure-Exp stream so the activation table never reloads);
           P^T @ V in bf16 accumulating over key chunks.
  norm     per pair: copy the two PSUM denominator rows into one [2, 512]
           tile, one DVE iterative reciprocal, a tiny ones-matmul broadcast,
           one [128, 512] DVE multiply -> outHT. All off the critical path
           (outHT is consumed one block later).
  proj     f32r, for block nb-1, interleaved 2 cout-chunks per pair into
           block nb's attention stream (dense PE, spread-out yT DMA).
"""

import sys

if "/opt/trn_rl_repo" not in sys.path:
    sys.path.insert(0, "/opt/trn_rl_repo")

import numpy as np

B, N, C = 4, 2048, 1024
H, HD = 16, 64
NCORES = 8
HLOC = H // 2          # heads per core
PAIRS = HLOC // 2      # head-pair tiles per core
CIN = HLOC * HD        # 512: proj input slice per core
NQB = 512              # query-block width
NBLK = N // NQB        # 4
CCH = C // 128         # 8 contraction chunks for the projections
KCH = N // 128         # 16 key chunks

MM_DT_NAME = "float32r"    # qkv + scores + proj matmul dtype
PV_DT_NAME = "bfloat16"    # v and exp(S) tiles (the P@V matmul dtype)

_BUILD_CACHE = {}


def _build(key):
    import concourse.mybir as mybir
    import concourse.tile as tile
    from concourse import bacc

    DT = getattr(mybir.dt, MM_DT_NAME)
    AT = getattr(mybir.dt, PV_DT_NAME)
    F32 = mybir.dt.float32
    AF = mybir.ActivationFunctionType

    nc = bacc.Bacc(None, target_bir_lowering=False)
    xT = nc.dram_tensor("xT", [C, N], DT, kind="ExternalInput")
    wqT = nc.dram_tensor("wqT", [C, CIN], DT, kind="ExternalInput")
    wkT = nc.dram_tensor("wkT", [C, CIN], DT, kind="ExternalInput")
    wvT = nc.dram_tensor("wvT", [C, CIN], DT, kind="ExternalInput")
    wpT = nc.dram_tensor("wpT", [CIN, C], DT, kind="ExternalInput")
    yT = nc.dram_tensor("yT", [C, N], F32, kind="ExternalOutput")

    with nc.allow_low_precision(reason="softmax intermediates kept in bf16"):
        with tile.TileContext(nc) as tc:
            _emit(nc, tc, tile, mybir, DT, AT, F32, AF, xT, wqT, wkT, wvT, wpT, yT)
    nc.compile()
    return nc


def _emit(nc, tc, tile, mybir, DT, AT, F32, AF, xT, wqT, wkT, wvT, wpT, yT):
    from contextlib import ExitStack

    ctx = ExitStack()
    with ctx:
        persist = ctx.enter_context(tc.tile_pool(name="persist", bufs=1))
        # wk chunks in pass A, then exp tiles (same 2KB/partition slot size)
        rot = ctx.enter_context(tc.tile_pool(name="rot", bufs=8))
        xpool = ctx.enter_context(tc.tile_pool(name="xpool", bufs=12))
        qpool = ctx.enter_context(tc.tile_pool(name="qpool", bufs=2))
        small = ctx.enter_context(tc.tile_pool(name="small", bufs=2))
        pvpool = ctx.enter_context(tc.tile_pool(name="pvpool", bufs=3))
        ypool = ctx.enter_context(tc.tile_pool(name="ypool", bufs=3))
        outs = ctx.enter_context(tc.tile_pool(name="outs", bufs=2))
        ps_s = ctx.enter_context(tc.tile_pool(name="ps_s", bufs=2, space="PSUM"))
        ps_v = ctx.enter_context(tc.tile_pool(name="ps_v", bufs=2, space="PSUM"))
        ps_acc = ctx.enter_context(tc.tile_pool(name="ps_acc", bufs=2, space="PSUM"))

        # --- persistent tiles ---------------------------------------------
        kT = [persist.tile([128, N], DT, tag=f"kT{p}", name=f"kT{p}") for p in range(PAIRS)]
        v_sb = [persist.tile([128, (HLOC + 1) * (HD + 1)], AT, tag=f"v{kc}", name=f"v{kc}") for kc in range(KCH)]
        wq_sb = [persist.tile([128, CIN], DT, tag=f"wq{ci}", name=f"wq{ci}") for ci in range(CCH)]
        wv_sb = [persist.tile([128, CIN], DT, tag=f"wv{ci}", name=f"wv{ci}") for ci in range(CCH)]
        wp_sb = [persist.tile([128, C], DT, tag=f"wp{pc}", name=f"wp{pc}") for pc in range(CIN // 128)]
        ones_m = persist.tile([1, HD], DT, tag="ones_m")  # bc-matmul stationary
        ones_f32 = persist.tile([128, HLOC], F32, tag="ones_f32")

        nc.vector.memset(ones_f32[:], 1.0)
        nc.vector.tensor_copy(ones_m[:], ones_f32[0:1, 0:1].broadcast_to((1, HD)))
        for kc in range(KCH):
            v3 = v_sb[kc][:, 0:HLOC * (HD + 1)].rearrange("p (h d) -> p h d", h=HLOC)
            nc.vector.tensor_copy(v3[:, :, HD], ones_f32[:, 0:HLOC])
            # zero tail pad so head 7's 128-wide stationary window reads zeros
            nc.vector.memset(v_sb[kc][:, HLOC * (HD + 1):].bitcast(mybir.dt.uint16), 0)

        def x_fetch(nb):
            xt = []
            for ci in range(CCH):
                t = xpool.tile([128, NQB], DT, tag="xt", name=f"xt{nb}_{ci}")
                nc.sync.dma_start(t[:], xT[ci * 128:(ci + 1) * 128, nb * NQB:(nb + 1) * NQB])
                xt.append(t)
            return xt

        xt_next = x_fetch(0)
        wk_sb = []
        for ci in range(CCH):
            w = rot.tile([128, CIN], DT, tag="rot", name=f"wk{ci}")
            nc.sync.dma_start(w[:], wkT[ci * 128:(ci + 1) * 128, :])
            wk_sb.append(w)
            nc.sync.dma_start(wv_sb[ci][:], wvT[ci * 128:(ci + 1) * 128, :])
        for ci in range(CCH):
            nc.sync.dma_start(wq_sb[ci][:], wqT[ci * 128:(ci + 1) * 128, :])
        for pc in range(CIN // 128):
            nc.sync.dma_start(wp_sb[pc][:], wpT[pc * 128:(pc + 1) * 128, :])

        # --- pass A: k and v for all query blocks -------------------------
        for nb in range(NBLK):
            nsl = slice(nb * NQB, (nb + 1) * NQB)
            xt = xt_next
            if nb + 1 < NBLK:
                xt_next = x_fetch(nb + 1)
            for dt_i in range(PAIRS):
                acc = ps_acc.tile([128, NQB], F32, tag="acc")
                for ci in range(CCH):
                    nc.tensor.matmul(
                        acc[:], wk_sb[ci][:, dt_i * 128:(dt_i + 1) * 128], xt[ci][:],
                        start=(ci == 0), stop=(ci == CCH - 1),
                    )
                nc.vector.tensor_copy(kT[dt_i][:, nsl], acc[:])
            for j in range(NQB // 128):
                kc = nb * (NQB // 128) + j
                acc = ps_acc.tile([128, CIN], F32, tag="acc")
                for ci in range(CCH):
                    nc.tensor.matmul(
                        acc[:], xt[ci][:, j * 128:(j + 1) * 128], wv_sb[ci][:],
                        start=(ci == 0), stop=(ci == CCH - 1),
                    )
                v3 = v_sb[kc][:, 0:HLOC * (HD + 1)].rearrange("p (h d) -> p h d", h=HLOC)
                nc.vector.tensor_copy(
                    v3[:, :, 0:HD],
                    acc[:].rearrange("p (h d) -> p h d", h=HLOC),
                )

        # q tiles are block-local and zero-padded per head (head's 64 dims on
        # its home partitions, zeros elsewhere) so the score matmul can use
        # the full [128, x] kT pair tile as stationary: full PE rows keep the
        # HAM activity monitor from re-throttling the clock. The pads are
        # zeroed once per ring buffer; in-loop writes only touch home rows.
        for gen in range(2):
            for h in range(HLOC):
                t = qpool.tile([128, NQB], DT, tag=f"qz{h}", name=f"qz{h}")
                pad = slice(64, 128) if h % 2 == 0 else slice(0, 64)
                nc.vector.memset(t[pad, :].bitcast(mybir.dt.uint32), 0)

        def q_group(xt, p):
            """q for pair p of one block -> two zero-padded qz tiles."""
            acc = ps_acc.tile([128, NQB], F32, tag="acc")
            for ci in range(CCH):
                nc.tensor.matmul(
                    acc[:], wq_sb[ci][:, p * 128:(p + 1) * 128], xt[ci][:],
                    start=(ci == 0), stop=(ci == CCH - 1),
                )
            ta = qpool.tile([128, NQB], DT, tag=f"qz{2 * p}", name=f"qz{2 * p}")
            tb = qpool.tile([128, NQB], DT, tag=f"qz{2 * p + 1}", name=f"qz{2 * p + 1}")
            nc.vector.tensor_copy(ta[0:64, :], acc[0:64, :])
            nc.vector.tensor_copy(tb[64:128, :], acc[64:128, :])
            return ta, tb

        # q for block 0 (its pass-A x chunks have rotated out; re-fetch)
        xtq = x_fetch(0)
        qz_next = []
        for p in range(PAIRS):
            qz_next.extend(q_group(xtq, p))

        # --- attention + projection, per query block ----------------------
        def proj_emit(nb_prev, outHT_prev):
            nsl_prev = slice(nb_prev * NQB, (nb_prev + 1) * NQB)
            for ct in range(C // 128):
                acc = ps_acc.tile([128, NQB], F32, tag="acc", name="acc")
                for p in range(PAIRS):
                    nc.tensor.matmul(
                        acc[:], wp_sb[p][:, ct * 128:(ct + 1) * 128],
                        outHT_prev[p][:],
                        start=(p == 0), stop=(p == PAIRS - 1),
                    )
                yt = ypool.tile([128, NQB], F32, tag="yt", name="yt")
                nc.vector.tensor_copy(yt[:], acc[:])
                nc.sync.dma_start(yT[ct * 128:(ct + 1) * 128, nsl_prev], yt[:])
                yield

        proj_gen = None
        for nb in range(NBLK):
            qz = qz_next
            if nb + 1 < NBLK:
                xtq = x_fetch(nb + 1)
            outHT = [outs.tile([128, NQB], DT, tag=f"outHT{p}", name=f"outHT{p}") for p in range(PAIRS)]
            qz_next = []
            for p in range(PAIRS):
                pv_a = ps_v.tile([128, NQB], F32, tag="pv", name="pv_a")
                pv_b = ps_v.tile([128, NQB], F32, tag="pv", name="pv_b")
                for kc2 in range(KCH // 2):
                    # issue order: 4x S, 2x exp, 4x V — the PE never has to
                    # wait mid-group on ScalarE (keeps HAM warm)
                    st_et = []
                    for head in range(2):
                        st = ps_s.tile([128, 2 * NQB], F32, tag="st", name="st")
                        et = rot.tile([128, 2 * NQB], AT, tag="rot", name="et")
                        st_et.append((st, et))
                    for half in range(2):
                        kc = kc2 * 2 + half
                        ksl = slice(kc * 128, (kc + 1) * 128)
                        csl = slice(half * NQB, (half + 1) * NQB)
                        for head in range(2):
                            nc.tensor.matmul(
                                st_et[head][0][:, csl],
                                kT[p][:, ksl],
                                qz[2 * p + head][:, :],
                                start=True, stop=True,
                            )
                    for st, et in st_et:
                        nc.scalar.activation(et[:], st[:], AF.Exp, scale=0.125)
                    for head, pv in ((0, pv_a), (1, pv_b)):
                        et = st_et[head][1]
                        vstart = (2 * p + head) * (HD + 1)
                        for half in range(2):
                            kc = kc2 * 2 + half
                            csl = slice(half * NQB, (half + 1) * NQB)
                            nc.tensor.matmul(
                                pv[:], v_sb[kc][:, vstart:vstart + 128], et[:, csl],
                                start=(kc == 0), stop=(kc == KCH - 1),
                            )

                # normalize: PSUM rows 0-63 are out^T, row 64 the denominator.
                # Copy PSUM out first (frees the pv accumulator banks quickly
                # so the next pair's P@V can start), then the reciprocal chain
                # (exact DVE iterative divide — ScalarE stays a pure-Exp
                # stream so its activation table never reloads) runs off the
                # critical path against the SBUF copy.
                pv_sbs = []
                for head, pv in ((0, pv_a), (1, pv_b)):
                    pv_sb = pvpool.tile([HD + 1, NQB], F32, tag="pv_sb", name="pv_sb")
                    nc.vector.tensor_copy(pv_sb[:], pv[0:HD + 1, :])
                    pv_sbs.append(pv_sb)
                for head, pv_sb, rbase in ((0, pv_sbs[0], 0), (1, pv_sbs[1], 64)):
                    rec = small.tile([1, NQB], F32, tag="rec", name="rec")
                    nc.vector.reciprocal(rec[:], pv_sb[HD:HD + 1, :])
                    rec_dt = small.tile([1, NQB], DT, tag="rec_dt", name="rec_dt")
                    nc.vector.tensor_copy(rec_dt[:], rec[:])
                    bc = ps_acc.tile([HD, NQB], F32, tag="acc", name="bc")
                    nc.tensor.matmul(bc[:], ones_m[:], rec_dt[:], start=True, stop=True)
                    nc.vector.tensor_mul(
                        outHT[p][rbase:rbase + HD, :], pv_sb[0:HD, :], bc[:],
                    )

                # interleaved work: 2 cout-chunks of block nb-1's projection,
                # then the q matmul group for pair p of block nb+1
                if proj_gen is not None:
                    next(proj_gen, None)
                    next(proj_gen, None)
                if nb + 1 < NBLK:
                    qz_next.extend(q_group(xtq, p))
            proj_gen = proj_emit(nb, outHT)
        for _ in proj_gen:
            pass


def _get_nc():
    key = (MM_DT_NAME, PV_DT_NAME)
    if key not in _BUILD_CACHE:
        _BUILD_CACHE[key] = _build(key)
    return _BUILD_CACHE[key]


def _make_in_maps(np_inputs):
    x = np.asarray(np_inputs["x"], dtype=np.float32)
    W_qkv = np.asarray(np_inputs["W_qkv"], dtype=np.float32)
    W_proj = np.asarray(np_inputs["W_proj"], dtype=np.float32)
    in_maps = []
    for c in range(NCORES):
        b, g = divmod(c, 2)
        rq = slice(g * CIN, (g + 1) * CIN)
        rk = slice(C + g * CIN, C + (g + 1) * CIN)
        rv = slice(2 * C + g * CIN, 2 * C + (g + 1) * CIN)
        in_maps.append({
            "xT": np.ascontiguousarray(x[b].T),
            "wqT": np.ascontiguousarray(W_qkv[rq].T),
            "wkT": np.ascontiguousarray(W_qkv[rk].T),
            "wvT": np.ascontiguousarray(W_qkv[rv].T),
            "wpT": np.ascontiguousarray(W_proj[:, g * CIN:(g + 1) * CIN].T),
        })
    return in_maps


def kernel(x, W_qkv, W_proj, b_proj):
    from concourse import bass_utils

    b_proj = np.asarray(b_proj, dtype=np.float32)
    nc = _get_nc()
    in_maps = _make_in_maps({"x": x, "W_qkv": W_qkv, "W_proj": W_proj})
    res = bass_utils.run_bass_kernel_spmd(nc, in_maps, core_ids=list(range(NCORES)))
    y = np.empty((B, N, C), dtype=np.float32)
    for b in range(B):
        yt = res.results[2 * b]["yT"] + res.results[2 * b + 1]["yT"]
        y[b] = yt.T
    return y + b_proj[None, None, :]
